# revision 1
# baseline (speedup 1.0000x reference)
"""Trainium2 Bass kernel for MetaPartModule (slot-attention style, 3 iterations).

Sharding: data-parallel over batch b (64) across 8 cores (8 batches/core).
BatchNorm statistics are made exact-global via two small AllReduces per
iteration (per-feature q stats, per-channel attn stats).

Per-core layout:
  - local batches grouped as 2 groups x 4 batches; packed partition row
    p = 32*bhat + i  (bhat = batch-in-group, i = slot index), bb = 4*g + bhat.
  - x needed in two majors: d-on-partitions ("xn", for attn = q @ k^T) and
    hw-on-partitions ("xt", for update = attn_n @ v). Both bf16; part
    resident in SBUF, rest streamed from HBM each iteration.
"""

import numpy as np
import ml_dtypes

import concourse.bass as bass
import concourse.tile as tile
from concourse import bacc, mybir
from concourse.bass_utils import run_bass_kernel_spmd
from concourse.masks import make_identity

F32 = mybir.dt.float32
BF16 = mybir.dt.bfloat16
F8 = mybir.dt.float8e4
AF = mybir.ActivationFunctionType
ALU = mybir.AluOpType
AX = mybir.AxisListType

N_CORES = 8
B = 64
C = 512
HW = 4096
N = 32
B_LOC = B // N_CORES      # 8
G = 2                     # batch groups per core
BH = 4                    # batches per group
DT = 4                    # d tiles of 128
JT = 32                   # hw tiles of 128
NT = 8                    # hw chunks of 512
ITERS = 3
ALPHA = 0.1
BN_EPS = 1e-5
EPS = 1e-12

NT_RES = 2                # resident hw chunks (of 512) for xn, per batch
JT_RES = 4                # resident hw tiles (of 128) for xt, per batch

_CACHE = {}


def _build(repeat=1):
    key = ("nc", repeat)
    if key in _CACHE:
        return _CACHE[key]
    nc = bacc.Bacc("TRN2", target_bir_lowering=False, debug=False,
                   num_devices=N_CORES)

    xn_h = nc.dram_tensor("xn", [B_LOC, C, HW], BF16, kind="ExternalInput")
    xt_h = nc.dram_tensor("xt", [G, JT, BH, 128, C], BF16, kind="ExternalInput")
    xn8_h = nc.dram_tensor("xn8", [B_LOC, C, HW], F8, kind="ExternalInput")
    xt8_h = nc.dram_tensor("xt8", [G, JT, BH, 128, C], F8, kind="ExternalInput")
    wq_h = nc.dram_tensor("wq", [128, DT, C], BF16, kind="ExternalInput")
    mT_h = nc.dram_tensor("mT", [128, DT, N], BF16, kind="ExternalInput")
    m128_h = nc.dram_tensor("m128", [128, C], F32, kind="ExternalInput")
    g1b1_h = nc.dram_tensor("g1b1", [128, DT, 2], F32, kind="ExternalInput")
    g2b2_h = nc.dram_tensor("g2b2", [128, 2], F32, kind="ExternalInput")
    fold_h = nc.dram_tensor("fold", [128, 128], F32, kind="ExternalInput")
    out_h = nc.dram_tensor("out", [G, 128, C], F32, kind="ExternalOutput")

    with tile.TileContext(nc) as tc:
        with (
            tc.tile_pool(name="const", bufs=1) as constp,
            tc.tile_pool(name="xres", bufs=1) as xres,
            tc.tile_pool(name="attn", bufs=1) as attnp,
            tc.tile_pool(name="state", bufs=1) as statep,
            tc.tile_pool(name="stats", bufs=2) as statsp,
            tc.tile_pool(name="small", bufs=2) as smallp,
            tc.tile_pool(name="xn_ring", bufs=2) as xnring,
            tc.tile_pool(name="xt_ring", bufs=2) as xtring,
            tc.tile_pool(name="dram", bufs=2, space="DRAM") as dram,
            tc.tile_pool(name="pa", bufs=2, space="PSUM") as pa,
            tc.tile_pool(name="pu", bufs=2, space="PSUM") as pu,
            tc.tile_pool(name="pq", bufs=2, space="PSUM") as pq,
            tc.tile_pool(name="ptr", bufs=1, space="PSUM") as ptr,
        ):
            # ---- constants ----
            wq_sb = constp.tile([128, DT, C], BF16)
            nc.sync.dma_start(wq_sb[:], wq_h[:])
            mT_sb = constp.tile([128, DT, N], BF16)
            nc.sync.dma_start(mT_sb[:], mT_h[:])
            g1b1_sb = constp.tile([128, DT, 2], F32)
            nc.sync.dma_start(g1b1_sb[:], g1b1_h[:])
            g2b2_sb = constp.tile([128, 2], F32)
            nc.sync.dma_start(g2b2_sb[:], g2b2_h[:])
            fold_sb = constp.tile([128, 128], F32)
            nc.sync.dma_start(fold_sb[:], fold_h[:])
            ident = constp.tile([128, 128], F32)
            make_identity(nc, ident[:])
            eps_sb = constp.tile([128, 1], F32)
            nc.gpsimd.memset(eps_sb[:], BN_EPS)

            # ---- state ----
            slots_a = statep.tile([128, G, C], F32, tag="slots_a")
            slots_b = statep.tile([128, G, C], F32, tag="slots_b")
            slots_ab = [slots_a, slots_b]
            for g in range(G):
                nc.sync.dma_start(slots_ab[0][:, g, :], m128_h[:])
            slotsT_sb = statep.tile([128, DT, G * 128], BF16)
            q_sb = statep.tile([128, DT, G * 128], F32)
            qbn_sb = statep.tile([128, DT, G * 128], BF16)
            out_stage = statep.tile([128, G, C], F32)

            # ---- attn working set ----
            attn_raw = attnp.tile([128, G, HW], BF16)
            attn_nT = attnp.tile([128, G, JT, 128], BF16)
            rs_sb = attnp.tile([128, G], F32)
            recip_sb = attnp.tile([128, G], F32)

            # ---- resident x ----
            xn_res = xres.tile([128, B_LOC, DT, NT_RES * 512], BF16)
            for bb in range(B_LOC):
                nc.sync.dma_start(
                    xn_res[:, bb, :, :],
                    xn_h[bb, :, 0:NT_RES * 512].rearrange(
                        "(dt p) w -> p dt w", p=128),
                )
            xt_res = xres.tile([128, B_LOC, JT_RES, C], BF16)
            for g in range(G):
                for bh in range(BH):
                      nc.scalar.dma_start(
                        xt_res[:, 4 * g + bh, :, :],
                        xt_h[g, 0:JT_RES, bh].rearrange("jt p d -> p jt d"),
                    )

            for rep in range(repeat):
              if rep > 0:
                # re-init slots so every repetition does identical work
                for g in range(G):
                    nc.sync.dma_start(slots_ab[0][:, g, :], m128_h[:])
              for t in range(ITERS):
                  # ============ Q phase ============
                  if t == 0:
                      # slots identical across batches: q over the 32 unique rows
                      nrow = N
                      q_rhs = lambda dt_i: mT_sb[:, dt_i, :]
                  else:
                      nrow = G * 128
                      # transpose slots (row-major packed) -> slotsT (d-major)
                      slots_cur = slots_ab[t % 2]
                      for g in range(G):
                          for dc in range(DT):
                              ps_tr = ptr.tile([128, 128], F32)
                              nc.tensor.transpose(
                                  ps_tr[:], slots_cur[:, g, dc * 128:(dc + 1) * 128],
                                  ident[:])
                              nc.vector.tensor_copy(
                                  slotsT_sb[:, dc, g * 128:(g + 1) * 128], ps_tr[:])
                      q_rhs = lambda dt_i: slotsT_sb[:, dt_i, :]

                  def warm_pe(nmm=28):
                      wps = pq.tile([128, G * 128], F32, tag="psq")
                      for _ in range(nmm):
                          nc.tensor.matmul(
                              wps[:, 0:256], wq_sb[:, 0, 0:128],
                              wq_sb[:, 0, 0:256])

                  qmv = statsp.tile([128, DT, 2], F32, tag="qmv")
                  for dt_o in range(DT):
                      ps_q = pq.tile([128, G * 128], F32, tag="psq")
                      for dt_i in range(DT):
                          nc.tensor.matmul(
                              ps_q[:, 0:nrow],
                              wq_sb[:, dt_i, dt_o * 128:(dt_o + 1) * 128],
                              q_rhs(dt_i),
                              start=(dt_i == 0), stop=(dt_i == DT - 1),
                          )
                      nc.vector.tensor_copy(q_sb[:, dt_o, 0:nrow], ps_q[:, 0:nrow])
                      bnst = statsp.tile([128, 6], F32, tag="bnst")
                      nc.vector.bn_stats(bnst[:], q_sb[:, dt_o, 0:nrow])
                      nc.vector.bn_aggr(qmv[:, dt_o, :], bnst[:])

                  a1_sb = statsp.tile([128, DT], F32, tag="a1")
                  c1_sb = statsp.tile([128, DT], F32, tag="c1")
                  tmp4 = statsp.tile([128, DT], F32, tag="tmp4")
                  sd4 = statsp.tile([128, DT], F32, tag="sd4")
                  if t == 0:
                      mu_ap = qmv[:, :, 0]
                      var_ap = qmv[:, :, 1]
                      nc.scalar.activation(sd4[:], var_ap, AF.Sqrt, bias=eps_sb[:])
                      nc.vector.reciprocal(a1_sb[:], sd4[:])
                      nc.vector.tensor_mul(a1_sb[:], a1_sb[:], g1b1_sb[:, :, 0])
                      nc.vector.tensor_mul(tmp4[:], mu_ap, a1_sb[:])
                      nc.vector.tensor_sub(c1_sb[:], g1b1_sb[:, :, 1], tmp4[:])
                  else:
                      # local (mean, E2) -> AllReduce -> global stats
                      arq = statsp.tile([128, DT, 2], F32, tag="arq")
                      nc.vector.tensor_copy(arq[:, :, 0], qmv[:, :, 0])
                      nc.vector.tensor_mul(tmp4[:], qmv[:, :, 0], qmv[:, :, 0])
                      nc.vector.tensor_add(arq[:, :, 1], qmv[:, :, 1], tmp4[:])
                      qcc_in = dram.tile([128, DT, 2], F32, tag="qcc_in")
                      qcc_out = dram.tile([128, DT, 2], F32, tag="qcc_out")
                      nc.sync.dma_start(qcc_in[:], arq[:])
                      nc.gpsimd.collective_compute(
                          "AllReduce", ALU.add,
                          replica_groups=[list(range(N_CORES))],
                          ins=[qcc_in.opt()], outs=[qcc_out.opt()],
                      )
                      warm_pe()
                      gq = statsp.tile([128, DT, 2], F32, tag="gq")
                      nc.sync.dma_start(gq[:], qcc_out[:])
                      mu4 = statsp.tile([128, DT], F32, tag="mu4")
                      nc.vector.tensor_scalar_mul(mu4[:], gq[:, :, 0], 1.0 / N_CORES)
                      e24 = statsp.tile([128, DT], F32, tag="e24")
                      nc.vector.tensor_scalar_mul(e24[:], gq[:, :, 1], 1.0 / N_CORES)
                      nc.vector.tensor_mul(tmp4[:], mu4[:], mu4[:])
                      nc.vector.tensor_sub(e24[:], e24[:], tmp4[:])  # var
                      nc.scalar.activation(sd4[:], e24[:], AF.Sqrt, bias=eps_sb[:])
                      nc.vector.reciprocal(a1_sb[:], sd4[:])
                      nc.vector.tensor_mul(a1_sb[:], a1_sb[:], g1b1_sb[:, :, 0])
                      nc.vector.tensor_mul(tmp4[:], mu4[:], a1_sb[:])
                      nc.vector.tensor_sub(c1_sb[:], g1b1_sb[:, :, 1], tmp4[:])

                  for dt_o in range(DT):
                      nc.scalar.activation(
                          qbn_sb[:, dt_o, 0:nrow], q_sb[:, dt_o, 0:nrow], AF.Relu,
                          scale=a1_sb[:, dt_o:dt_o + 1], bias=c1_sb[:, dt_o:dt_o + 1])

                  # ============ ATTN phase ============
                  bnsta = statsp.tile([128, G * NT, 6], F32, tag="bnsta")
                  nt_order = list(range(NT_RES, NT)) + list(range(NT_RES))
                  for g in range(G):
                      for nt in nt_order:
                          if nt < NT_RES:
                              rhs = lambda bh, dt: xn_res[
                                  :, 4 * g + bh, dt, nt * 512:(nt + 1) * 512]
                          else:
                              s_dt = F8 if t < ITERS - 1 else BF16
                              s_h = xn8_h if t < ITERS - 1 else xn_h
                              rt = xnring.tile([128, BH, DT, 512], s_dt, tag="xnr")
                              nc.sync.dma_start(
                                  rt[:],
                                  s_h[4 * g:4 * g + 4, :, nt * 512:(nt + 1) * 512]
                                  .rearrange("b (dt p) w -> p b dt w", p=128),
                              )
                              rhs = lambda bh, dt, rt=rt: rt[:, bh, dt, :]
                          ps_a = pa.tile([128, 512], F32, tag="psa")
                          for dt in range(DT):
                              for bh in range(BH):
                                  if t == 0:
                                      lhsT = qbn_sb[:, dt, 0:N]
                                  else:
                                      lhsT = qbn_sb[:, dt,
                                                    g * 128 + 32 * bh:
                                                    g * 128 + 32 * bh + 32]
                                  nc.tensor.matmul(
                                      ps_a[32 * bh:32 * bh + 32, :],
                                      lhsT, rhs(bh, dt),
                                      start=(dt == 0), stop=(dt == DT - 1),
                                      tile_position=(0, 32 * bh),
                                  )
                          nc.vector.bn_stats(bnsta[:, g * NT + nt, :], ps_a[:])
                          nc.scalar.copy(
                              attn_raw[:, g, nt * 512:(nt + 1) * 512], ps_a[:])

                  amv = statsp.tile([128, 2], F32, tag="amv")
                  nc.vector.bn_aggr(amv[:], bnsta[:])
                  ar2 = statsp.tile([128, 2], F32, tag="ar2")
                  tmp1 = statsp.tile([128, 1], F32, tag="tmp1")
                  nc.vector.tensor_copy(ar2[:, 0:1], amv[:, 0:1])
                  nc.vector.tensor_mul(tmp1[:], amv[:, 0:1], amv[:, 0:1])
                  nc.vector.tensor_add(ar2[:, 1:2], amv[:, 1:2], tmp1[:])
                  acc_in = dram.tile([128, 2], F32, tag="acc_in")
                  acc_out = dram.tile([128, 2], F32, tag="acc_out")
                  nc.sync.dma_start(acc_in[:], ar2[:])
                  nc.gpsimd.collective_compute(
                      "AllReduce", ALU.add,
                      replica_groups=[list(range(N_CORES))],
                      ins=[acc_in.opt()], outs=[acc_out.opt()],
                  )
                  warm_pe()
                  ga = statsp.tile([128, 2], F32, tag="ga")
                  nc.sync.dma_start(ga[:], acc_out[:])

                  # fold stats across the 4 bhat blocks (and broadcast to all
                  # partitions of the same channel) with one masked matmul
                  ps_f = ptr.tile([128, 2], F32, tag="psf")
                  nc.tensor.matmul(ps_f[:], fold_sb[:], ga[:])
                  mu1 = smallp.tile([128, 1], F32, tag="mu1")
                  nc.vector.tensor_scalar_mul(mu1[:], ps_f[:, 0:1], 1.0 / (4 * N_CORES))
                  e21 = smallp.tile([128, 1], F32, tag="e21")
                  nc.vector.tensor_scalar_mul(e21[:], ps_f[:, 1:2], 1.0 / (4 * N_CORES))
                  var1 = smallp.tile([128, 1], F32, tag="var1")
                  nc.vector.tensor_mul(var1[:], mu1[:], mu1[:])
                  nc.vector.tensor_sub(var1[:], e21[:], var1[:])
                  sd1 = smallp.tile([128, 1], F32, tag="sd1")
                  nc.scalar.activation(sd1[:], var1[:], AF.Sqrt, bias=eps_sb[:])
                  ac128 = smallp.tile([128, 2], F32, tag="ac128")
                  nc.vector.reciprocal(ac128[:, 0:1], sd1[:])
                  nc.vector.tensor_mul(ac128[:, 0:1], ac128[:, 0:1], g2b2_sb[:, 0:1])
                  nc.vector.tensor_mul(ac128[:, 1:2], mu1[:], ac128[:, 0:1])
                  nc.vector.tensor_sub(ac128[:, 1:2], g2b2_sb[:, 1:2], ac128[:, 1:2])

                  # normalize + relu + rowsum, then transpose attn_n
                  for g in range(G):
                      nc.scalar.activation(
                          attn_raw[:, g, :], attn_raw[:, g, :], AF.Relu,
                          scale=ac128[:, 0:1], bias=ac128[:, 1:2],
                          accum_out=rs_sb[:, g:g + 1])
                      nc.sync.dma_start_transpose(
                          attn_nT[:, g, :, :], attn_raw[:, g, :])
                  nc.vector.tensor_scalar_add(recip_sb[:], rs_sb[:], EPS)
                  nc.vector.reciprocal(recip_sb[:], recip_sb[:])

                  # ============ UPDATE phase ============
                  jt_order = list(range(JT_RES, JT)) + list(range(JT_RES))
                  for g in range(G):
                      ps_u = pu.tile([128, C], F32, tag="psu")
                      for ji, jt in enumerate(jt_order):
                          if jt < JT_RES:
                              rhs_u = lambda bh, jt=jt: xt_res[:, 4 * g + bh, jt, :]
                          elif (jt - JT_RES) % 2 == 0:
                              s_dt = F8 if t < ITERS - 1 else BF16
                              s_h = xt8_h if t < ITERS - 1 else xt_h
                              ut = xtring.tile([128, 2, BH, C], s_dt, tag="xtr")
                              nc.scalar.dma_start(
                                  ut[:],
                                  s_h[g, jt:jt + 2].rearrange(
                                      "j b p d -> p j b d"),
                              )
                              rhs_u = lambda bh, ut=ut: ut[:, 0, bh, :]
                          else:
                              rhs_u = lambda bh, ut=ut: ut[:, 1, bh, :]
                          for bh in range(BH):
                              nc.tensor.matmul(
                                  ps_u[32 * bh:32 * bh + 32, :],
                                  attn_nT[:, g, jt, 32 * bh:32 * bh + 32],
                                  rhs_u(bh),
                                  start=(ji == 0), stop=(ji == JT - 1),
                                  tile_position=(0, 32 * bh),
                              )
                      if t < ITERS - 1:
                          coef = ALPHA / (1.0 - ALPHA) ** (t + 1)
                          rc = smallp.tile([128, 1], F32, tag="rc")
                          nc.vector.tensor_scalar_mul(
                              rc[:], recip_sb[:, g:g + 1], coef)
                          nc.vector.scalar_tensor_tensor(
                              slots_ab[(t + 1) % 2][:, g, :], ps_u[:], rc[:],
                              slots_ab[t % 2][:, g, :],
                              op0=ALU.mult, op1=ALU.add)
                      else:
                          nc.vector.tensor_scalar(
                              out_stage[:, g, :], ps_u[:], recip_sb[:, g:g + 1],
                              None, op0=ALU.mult)

            for g in range(G):
                nc.sync.dma_start(out_h[g], out_stage[:, g, :])

    nc.compile()
    _CACHE[key] = nc
    return nc


def _prep_inputs(x, meta_embed, Wq, g1, b1, g2, b2):
    bf16 = ml_dtypes.bfloat16
    x3 = np.asarray(x, dtype=np.float32).reshape(B, C, HW)
    Wq = np.asarray(Wq, dtype=np.float32)
    meta = np.asarray(meta_embed, dtype=np.float32)
    wq_t = np.ascontiguousarray(
        Wq.T.reshape(DT, 128, C).transpose(1, 0, 2)).astype(bf16)
    mT_t = np.ascontiguousarray(
        meta.T.reshape(DT, 128, N).transpose(1, 0, 2)).astype(bf16)
    m128 = np.ascontiguousarray(np.tile(meta, (BH, 1))).astype(np.float32)
    g1b1 = np.stack([np.asarray(g1, np.float32).reshape(DT, 128).T,
                     np.asarray(b1, np.float32).reshape(DT, 128).T], axis=-1)
    g1b1 = np.ascontiguousarray(g1b1)
    g2b2 = np.stack([np.tile(np.asarray(g2, np.float32), BH),
                     np.tile(np.asarray(b2, np.float32), BH)], axis=-1)
    g2b2 = np.ascontiguousarray(g2b2)
    fold = np.tile(np.eye(N, dtype=np.float32), (BH, BH))
    fold = np.ascontiguousarray(fold)

    in_maps = []
    for c in range(N_CORES):
        sl = x3[c * B_LOC:(c + 1) * B_LOC]
        f8 = ml_dtypes.float8_e4m3
        xn = np.ascontiguousarray(sl).astype(bf16)
        xt4 = sl.transpose(0, 2, 1).reshape(G, BH, JT, 128, C)
        xt = np.ascontiguousarray(xt4.transpose(0, 2, 1, 3, 4)).astype(bf16)
        in_maps.append({
            "xn": xn, "xt": xt, "xn8": xn.astype(f8), "xt8": xt.astype(f8),
            "wq": wq_t, "mT": mT_t, "m128": m128,
            "g1b1": g1b1, "g2b2": g2b2, "fold": fold,
        })
    return in_maps


def _unpack(results):
    out = np.empty((B, N, C), dtype=np.float32)
    for c in range(N_CORES):
        r = results[c]["out"]          # [G, 128, C]
        for g in range(G):
            blk = r[g].reshape(BH, N, C)
            for bh in range(BH):
                out[c * B_LOC + g * BH + bh] = blk[bh]
    return out


def run(trace=False, **inputs):
    nc = _build()
    in_maps = _prep_inputs(
        inputs["x"], inputs["meta_embed"], inputs["Wq"],
        inputs["g1"], inputs["b1"], inputs["g2"], inputs["b2"])
    res = run_bass_kernel_spmd(nc, in_maps, core_ids=list(range(N_CORES)),
                               trace=trace)
    return _unpack(res.results), res


def kernel(**inputs):
    out, _ = run(trace=False, **inputs)
    return out



# revision 17
# speedup vs baseline: 1.0105x; 1.0105x over previous
"""Trainium2 Bass kernel for MetaPartModule (slot-attention style, 3 iterations).

Sharding: data-parallel over batch b (64) across 8 cores (8 batches/core).
BatchNorm statistics exact-global via two small AllReduces per iteration.

v2 design (fp8 DoubleRow + hi/lo residual precision):
  - x kept resident in SBUF as fp8 e4m3 ("hi") in d-major (xn) for the attn
    phase of all 3 iterations; hw-major (xt) hi streamed from HBM per iter
    for the update phase.
  - Final iteration restores bf16-class accuracy with fp8 "lo" residuals
    (lo = e4m3((x - hi) * 16)): attn += q_hi*k_lo + q_lo*k_hi, update
    U += w_hi*v_lo + w_lo*v_hi, lo partial sums in a separate PSUM plane
    combined at 1/16 scale.
  - All attn/update matmuls run in fp8 DoubleRow mode (2 k-tiles per
    instruction, 0.5 cycles/row).
  - attn weights quantized to fp8 post BN+ReLU and transposed on the PE
    (128x128 fp8 tiles) to feed the update matmul as stationary operand.
  - Two-pass attn: pass 1 computes hi*hi products for BN stats only; pass 2
    (overlapping the stats AllReduce) recomputes products and applies
    normalize+quantize+transpose. Raw attn is never materialized in SBUF.

Per-core layout: 8 batches as 2 groups x 4 batches; packed partition row
p = 32*bh + i (bh = batch-in-group, i = slot index).
"""

import numpy as np
import ml_dtypes

import concourse.bass as bass
import concourse.tile as tile
from concourse import bacc, mybir
from concourse.bass_utils import run_bass_kernel_spmd
from concourse.masks import make_identity

F32 = mybir.dt.float32
BF16 = mybir.dt.bfloat16
F8 = mybir.dt.float8e4
AF = mybir.ActivationFunctionType
ALU = mybir.AluOpType
AX = mybir.AxisListType
DRow = mybir.MatmulPerfMode.DoubleRow

N_CORES = 8
B = 64
C = 512
HW = 4096
N = 32
B_LOC = B // N_CORES      # 8
G = 2                     # batch groups per core
BH = 4                    # batches per group
DT = 4                    # d tiles of 128
DTP = 2                   # d tile pairs (DoubleRow)
NT = 8                    # hw chunks of 512 (attn)
JT = 32                   # hw tiles of 128 (update)
JTP = 16                  # hw tile pairs of 256 (DoubleRow)
ITERS = 3
ALPHA = 0.1
BN_EPS = 1e-5
EPS = 1e-12
SLO = 16.0                # lo-residual scale
ISLO = 1.0 / SLO

_CACHE = {}


def _build(repeat=1):
    key = ("nc", repeat)
    if key in _CACHE:
        return _CACHE[key]
    nc = bacc.Bacc("TRN2", target_bir_lowering=False, debug=False,
                   num_devices=N_CORES)

    xnhi_h = nc.dram_tensor("xnhi", [NT, 128, B_LOC, DTP, 1024], F8,
                            kind="ExternalInput")
    xnlo_h = nc.dram_tensor("xnlo", [NT, DTP, G, 128, BH, 2, 512], F8,
                            kind="ExternalInput")
    xthi_h = nc.dram_tensor("xthi", [G, JTP, 128, BH, 2, 512], F8,
                            kind="ExternalInput")
    xtlo_h = nc.dram_tensor("xtlo", [G, JTP, 128, BH, 2, 512], F8,
                            kind="ExternalInput")
    wq_h = nc.dram_tensor("wq", [128, DT, C], BF16, kind="ExternalInput")
    mT_h = nc.dram_tensor("mT", [128, DT, N], BF16, kind="ExternalInput")
    m128_h = nc.dram_tensor("m128", [128, C], F32, kind="ExternalInput")
    g1b1_h = nc.dram_tensor("g1b1", [128, DT, 2], F32, kind="ExternalInput")
    g2b2_h = nc.dram_tensor("g2b2", [128, 2], F32, kind="ExternalInput")
    fold_h = nc.dram_tensor("fold", [128, 128], F32, kind="ExternalInput")
    out_h = nc.dram_tensor("out", [G, 128, C], F32, kind="ExternalOutput")

    with tile.TileContext(nc) as tc:
        with (
            tc.tile_pool(name="const", bufs=1) as constp,
            tc.tile_pool(name="xres", bufs=1) as xres,
            tc.tile_pool(name="attn", bufs=1) as attnp,
            tc.tile_pool(name="state", bufs=1) as statep,
            tc.tile_pool(name="stats", bufs=2) as statsp,
            tc.tile_pool(name="chunk", bufs=2) as chunkp,
            tc.tile_pool(name="small", bufs=2) as smallp,
            tc.tile_pool(name="xn_ring", bufs=2) as xnring,
            tc.tile_pool(name="xt_ring", bufs=3) as xtring,
            tc.tile_pool(name="xtl_ring", bufs=2) as xtlring,
            tc.tile_pool(name="dram", bufs=2, space="DRAM") as dram,
            tc.tile_pool(name="pa", bufs=2, space="PSUM") as pa,
            tc.tile_pool(name="palo", bufs=1, space="PSUM") as palo,
            tc.tile_pool(name="pu", bufs=1, space="PSUM") as pu,
            tc.tile_pool(name="pq", bufs=1, space="PSUM") as pq,
            tc.tile_pool(name="ptr", bufs=2, space="PSUM") as ptr,
        ):
            # ---- constants ----
            wq_sb = constp.tile([128, DT, C], BF16)
            nc.scalar.dma_start(wq_sb[:], wq_h[:])
            mT_sb = constp.tile([128, DT, N], BF16)
            nc.scalar.dma_start(mT_sb[:], mT_h[:])
            g1b1_sb = constp.tile([128, DT, 2], F32)
            nc.scalar.dma_start(g1b1_sb[:], g1b1_h[:])
            g2b2_sb = constp.tile([128, 2], F32)
            nc.scalar.dma_start(g2b2_sb[:], g2b2_h[:])
            fold_sb = constp.tile([128, 128], F32)
            nc.scalar.dma_start(fold_sb[:], fold_h[:])
            ident = constp.tile([128, 128], F32)
            make_identity(nc, ident[:])
            identb = constp.tile([128, 128], BF16)
            nc.scalar.copy(identb[:], ident[:])
            eps_sb = constp.tile([128, 1], F32)
            nc.gpsimd.memset(eps_sb[:], BN_EPS)

            # ---- resident x (d-major, fp8 hi) ----
            # [p, b, dtp, nt*1024 + kt*512 + w] = hi[b, (2dtp+kt)*128+p, nt*512+w]
            xn_res = xres.tile([128, B_LOC, DTP, NT * 1024], F8)
            for nt in range(NT):
                nc.scalar.dma_start(
                    xn_res[:, :, :, nt * 1024:(nt + 1) * 1024], xnhi_h[nt])

            # ---- state ----
            slots_a = statep.tile([128, G, C], F32, tag="slots_a")
            slots_b = statep.tile([128, G, C], F32, tag="slots_b")
            slots_ab = [slots_a, slots_b]
            for g in range(G):
                nc.scalar.dma_start(slots_ab[0][:, g, :], m128_h[:])
            slotsT_sb = statep.tile([128, DT, G * 128], BF16)
            q_sb = statep.tile([128, DT, G * 128], F32)
            qbn_hi = statep.tile([128, DT, G * 128], F8)
            qbn_lo = statep.tile([128, DT, G * 128], F8)

            # ---- attn working set ----
            attn_nT_hi = attnp.tile([128, G, JT, 128], F8)
            attn_nT_lo = attnp.tile([128, G, JT, 128], F8)
            rs_parts = attnp.tile([128, G, NT], F32)
            rs_sb = attnp.tile([128, G], F32)
            recip_sb = attnp.tile([128, G], F32)

            def warm_pe(nmm):
                wps = pq.tile([128, G * 128], F32, tag="psq")
                for _ in range(nmm):
                    nc.tensor.matmul(
                        wps[:, 0:256], wq_sb[:, 0, 0:128], wq_sb[:, 0, 0:256])

            def attn_mm(ps_plane, g, bh, t, lhsT_tile, rhs_of_dt, first, last):
                rows = (slice(0, 32) if t == 0 else
                        slice(g * 128 + 32 * bh, g * 128 + 32 * bh + 32))
                for dt in range(DT):
                    nc.tensor.matmul(
                        ps_plane[32 * bh:32 * bh + 32, :],
                        lhsT_tile[:, dt, rows],
                        rhs_of_dt(dt),
                        start=(first and dt == 0),
                        stop=(last and dt == DT - 1),
                        tile_position=(0, 32 * bh),
                    )

            def xn_rhs(bb, nt):
                # dt = (dtp, kt): xn_res free index = dtp-dim, kt*512 + w
                return lambda dt: xn_res[
                    :, bb, dt // 2,
                    nt * 1024 + (dt % 2) * 512:nt * 1024 + (dt % 2) * 512 + 512]

            def emit_q_phase(t):
                final = (t == ITERS - 1)
                if t == 0:
                    nrow = N
                    q_rhs = lambda dt_i: mT_sb[:, dt_i, :]
                else:
                    nrow = G * 128
                    slots_cur = slots_ab[t % 2]
                    for g in range(G):
                        for dc in range(DT):
                            ps_tr = ptr.tile([128, 128], F32, tag="ptrx")
                            nc.tensor.transpose(
                                ps_tr[:], slots_cur[:, g, dc * 128:(dc + 1) * 128],
                                ident[:])
                            nc.vector.tensor_copy(
                                slotsT_sb[:, dc, g * 128:(g + 1) * 128], ps_tr[:])
                    q_rhs = lambda dt_i: slotsT_sb[:, dt_i, :]

                qmv = statsp.tile([128, DT, 2], F32, tag="qmv")
                for dt_o in range(DT):
                    ps_q = pq.tile([128, G * 128], F32, tag="psq")
                    for dt_i in range(DT):
                        nc.tensor.matmul(
                            ps_q[:, 0:nrow],
                            wq_sb[:, dt_i, dt_o * 128:(dt_o + 1) * 128],
                            q_rhs(dt_i),
                            start=(dt_i == 0), stop=(dt_i == DT - 1),
                        )
                    nc.vector.tensor_copy(q_sb[:, dt_o, 0:nrow], ps_q[:, 0:nrow])
                    bnst = statsp.tile([128, 6], F32, tag="bnst")
                    nc.vector.bn_stats(bnst[:], q_sb[:, dt_o, 0:nrow])
                    nc.vector.bn_aggr(qmv[:, dt_o, :], bnst[:])

                a1_sb = statsp.tile([128, DT], F32, tag="a1")
                c1_sb = statsp.tile([128, DT], F32, tag="c1")
                tmp4 = statsp.tile([128, DT], F32, tag="tmp4")
                sd4 = statsp.tile([128, DT], F32, tag="sd4")
                if t == 0:
                    # slots identical across batches -> local stats are exact
                    nc.scalar.activation(sd4[:], qmv[:, :, 1], AF.Sqrt,
                                         bias=eps_sb[:])
                    nc.vector.reciprocal(a1_sb[:], sd4[:])
                    nc.vector.tensor_mul(a1_sb[:], a1_sb[:], g1b1_sb[:, :, 0])
                    nc.vector.tensor_mul(tmp4[:], qmv[:, :, 0], a1_sb[:])
                    nc.vector.tensor_sub(c1_sb[:], g1b1_sb[:, :, 1], tmp4[:])
                else:
                    # local (mean, E2) -> AllReduce -> global stats
                    arq = statsp.tile([128, DT, 2], F32, tag="arq")
                    nc.vector.tensor_copy(arq[:, :, 0], qmv[:, :, 0])
                    nc.vector.tensor_mul(tmp4[:], qmv[:, :, 0], qmv[:, :, 0])
                    nc.vector.tensor_add(arq[:, :, 1], qmv[:, :, 1], tmp4[:])
                    qcc_in = dram.tile([128, DT, 2], F32, tag="qcc_in")
                    qcc_out = dram.tile([128, DT, 2], F32, tag="qcc_out")
                    nc.scalar.dma_start(qcc_in[:], arq[:])
                    nc.gpsimd.collective_compute(
                        "AllReduce", ALU.add,
                        replica_groups=[list(range(N_CORES))],
                        ins=[qcc_in.opt()], outs=[qcc_out.opt()],
                    )
                    warm_pe(28)
                    gq = statsp.tile([128, DT, 2], F32, tag="gq")
                    nc.scalar.dma_start(gq[:], qcc_out[:])
                    mu4 = statsp.tile([128, DT], F32, tag="mu4")
                    nc.vector.tensor_scalar_mul(mu4[:], gq[:, :, 0], 1.0 / N_CORES)
                    e24 = statsp.tile([128, DT], F32, tag="e24")
                    nc.vector.tensor_scalar_mul(e24[:], gq[:, :, 1], 1.0 / N_CORES)
                    nc.vector.tensor_mul(tmp4[:], mu4[:], mu4[:])
                    nc.vector.tensor_sub(e24[:], e24[:], tmp4[:])  # var
                    nc.scalar.activation(sd4[:], e24[:], AF.Sqrt, bias=eps_sb[:])
                    nc.vector.reciprocal(a1_sb[:], sd4[:])
                    nc.vector.tensor_mul(a1_sb[:], a1_sb[:], g1b1_sb[:, :, 0])
                    nc.vector.tensor_mul(tmp4[:], mu4[:], a1_sb[:])
                    nc.vector.tensor_sub(c1_sb[:], g1b1_sb[:, :, 1], tmp4[:])

                for dt_o in range(DT):
                    nc.scalar.activation(
                        qbn_hi[:, dt_o, 0:nrow], q_sb[:, dt_o, 0:nrow], AF.Relu,
                        scale=a1_sb[:, dt_o:dt_o + 1], bias=c1_sb[:, dt_o:dt_o + 1])
                if final:
                    for dt_o in range(DT):
                        qf = chunkp.tile([128, G * 128], BF16, tag="qfc")
                        nc.scalar.activation(
                            qf[:, 0:nrow], q_sb[:, dt_o, 0:nrow], AF.Relu,
                            scale=a1_sb[:, dt_o:dt_o + 1],
                            bias=c1_sb[:, dt_o:dt_o + 1])
                        nc.vector.tensor_sub(
                            q_sb[:, dt_o, 0:nrow], qf[:, 0:nrow],
                            qbn_hi[:, dt_o, 0:nrow])
                        nc.scalar.activation(
                            qbn_lo[:, dt_o, 0:nrow], q_sb[:, dt_o, 0:nrow],
                            AF.Copy, scale=SLO)

            def emit_attn_pass1(t):
                bnsta = statsp.tile([128, G * NT, 6], F32, tag="bnsta")
                for g in range(G):
                    for nt in range(NT):
                        ps = pa.tile([128, 512], F32, tag="psa")
                        for bh in range(BH):
                            attn_mm(ps[:], g, bh, t, qbn_hi,
                                    xn_rhs(4 * g + bh, nt), True, True)
                        nc.vector.bn_stats(bnsta[:, g * NT + nt, :], ps[:])

                amv = statsp.tile([128, 2], F32, tag="amv")
                nc.vector.bn_aggr(amv[:], bnsta[:])
                ar2 = statsp.tile([128, 2], F32, tag="ar2")
                tmp1 = statsp.tile([128, 1], F32, tag="tmp1")
                nc.vector.tensor_copy(ar2[:, 0:1], amv[:, 0:1])
                nc.vector.tensor_mul(tmp1[:], amv[:, 0:1], amv[:, 0:1])
                nc.vector.tensor_add(ar2[:, 1:2], amv[:, 1:2], tmp1[:])
                acc_in = dram.tile([128, 2], F32, tag="acc_in")
                acc_out = dram.tile([128, 2], F32, tag="acc_out")
                nc.scalar.dma_start(acc_in[:], ar2[:])
                nc.gpsimd.collective_compute(
                    "AllReduce", ALU.add,
                    replica_groups=[list(range(N_CORES))],
                    ins=[acc_in.opt()], outs=[acc_out.opt()],
                )
                ga = statsp.tile([128, 2], F32, tag="ga")
                nc.scalar.dma_start(ga[:], acc_out[:])

                # fold stats across the 4 bhat blocks with one masked matmul
                ps_fq = pq.tile([128, G * 128], F32, tag="psq")
                ps_f = ps_fq[:, 0:2]
                nc.tensor.matmul(ps_f, fold_sb[:], ga[:])
                mu1 = smallp.tile([128, 1], F32, tag="mu1")
                nc.vector.tensor_scalar_mul(mu1[:], ps_f[:, 0:1],
                                            1.0 / (4 * N_CORES))
                e21 = smallp.tile([128, 1], F32, tag="e21")
                nc.vector.tensor_scalar_mul(e21[:], ps_f[:, 1:2],
                                            1.0 / (4 * N_CORES))
                var1 = smallp.tile([128, 1], F32, tag="var1")
                nc.vector.tensor_mul(var1[:], mu1[:], mu1[:])
                nc.vector.tensor_sub(var1[:], e21[:], var1[:])
                sd1 = smallp.tile([128, 1], F32, tag="sd1")
                nc.scalar.activation(sd1[:], var1[:], AF.Sqrt, bias=eps_sb[:])
                ac128 = smallp.tile([128, 2], F32, tag="ac128")
                nc.vector.reciprocal(ac128[:, 0:1], sd1[:])
                nc.vector.tensor_mul(ac128[:, 0:1], ac128[:, 0:1], g2b2_sb[:, 0:1])
                nc.vector.tensor_mul(ac128[:, 1:2], mu1[:], ac128[:, 0:1])
                nc.vector.tensor_sub(ac128[:, 1:2], g2b2_sb[:, 1:2], ac128[:, 1:2])
                return ac128

            def emit_attn_chunk2(t, g, nt, ac128):
                final = (t == ITERS - 1)
                ps = pa.tile([128, 512], F32, tag="psa")
                for bh in range(BH):
                    attn_mm(ps[:], g, bh, t, qbn_hi,
                            xn_rhs(4 * g + bh, nt), True, True)
                ps_lo = None
                if final:
                    ps_lo = palo.tile([128, 512], F32, tag="psal")
                    xl0 = xnring.tile([128, BH, 2, 512], F8, tag="xnl")
                    nc.sync.dma_start(xl0[:], xnlo_h[nt, 0, g])
                    xl1 = xnring.tile([128, BH, 2, 512], F8, tag="xnl")
                    nc.sync.dma_start(xl1[:], xnlo_h[nt, 1, g])
                    xls = [xl0, xl1]
                    for bh in range(BH):
                        # q_hi * k_lo
                        attn_mm(ps_lo[:], g, bh, t, qbn_hi,
                                lambda dt, bh=bh: xls[dt // 2][:, bh, dt % 2, :],
                                True, False)
                        # q_lo * k_hi
                        attn_mm(ps_lo[:], g, bh, t, qbn_lo,
                                xn_rhs(4 * g + bh, nt), False, True)
                if g == 0 and nt == 1:
                    warm_pe(16)
                # normalize (+ lo split) + rowsum; fp8 quantization happens in
                # the PSUM->SBUF cast after the bf16 PE transpose.
                if not final:
                    wb = chunkp.tile([128, 512], BF16, tag="wb")
                    nc.scalar.activation(
                        wb[:], ps[:], AF.Relu,
                        scale=ac128[:, 0:1], bias=ac128[:, 1:2],
                        accum_out=rs_parts[:, g, nt:nt + 1])
                    wlb = None
                else:
                    tl = chunkp.tile([128, 512], BF16, tag="tl")
                    nc.scalar.activation(tl[:], ps_lo[:], AF.Copy, scale=ISLO)
                    raw = chunkp.tile([128, 512], BF16, tag="raw")
                    nc.vector.tensor_add(raw[:], ps[:], tl[:])
                    wb = chunkp.tile([128, 512], BF16, tag="wb")
                    nc.scalar.activation(
                        wb[:], raw[:], AF.Relu,
                        scale=ac128[:, 0:1], bias=ac128[:, 1:2],
                        accum_out=rs_parts[:, g, nt:nt + 1])
                    whi = chunkp.tile([128, 512], F8, tag="whi")
                    nc.scalar.activation(whi[:], wb[:], AF.Copy)
                    dlo = chunkp.tile([128, 512], BF16, tag="tl")
                    nc.vector.tensor_sub(dlo[:], wb[:], whi[:])
                    wlb = chunkp.tile([128, 512], BF16, tag="wlb")
                    nc.scalar.activation(wlb[:], dlo[:], AF.Copy, scale=SLO)
                for j4 in range(4):
                    jt = nt * 4 + j4
                    pt = ptr.tile([128, 128], BF16, tag="ptrx")
                    nc.tensor.transpose(
                        pt[:], wb[:, j4 * 128:(j4 + 1) * 128], identb[:])
                    if j4 % 2 == 0:
                        nc.vector.tensor_copy(attn_nT_hi[:, g, jt, :], pt[:])
                    else:
                        nc.scalar.copy(attn_nT_hi[:, g, jt, :], pt[:])
                    if final:
                        pt2 = ptr.tile([128, 128], BF16, tag="ptrx")
                        nc.tensor.transpose(
                            pt2[:], wlb[:, j4 * 128:(j4 + 1) * 128], identb[:])
                        if j4 % 2 == 0:
                            nc.scalar.copy(attn_nT_lo[:, g, jt, :], pt2[:])
                        else:
                            nc.vector.tensor_copy(attn_nT_lo[:, g, jt, :], pt2[:])

            def emit_update_group(t, g):
                final = (t == ITERS - 1)
                ps_u = pu.tile([128, 2, 512], F32, tag="psu")
                for jtp in range(JTP):
                    vt = xtring.tile([128, BH, 2, 512], F8, tag="xtt")
                    nc.sync.dma_start(vt[:], xthi_h[g, jtp])
                    vl = None
                    if final:
                        vl = xtlring.tile([128, BH, 2, 512], F8, tag="xtl")
                        nc.scalar.dma_start(vl[:], xtlo_h[g, jtp])
                    for bh in range(BH):
                        for kt in range(2):
                            jt = 2 * jtp + kt
                            lhsT_hi = attn_nT_hi[:, g, jt, 32 * bh:32 * bh + 32]
                            nc.tensor.matmul(
                                ps_u[32 * bh:32 * bh + 32, 0, :],
                                lhsT_hi, vt[:, bh, kt, :],
                                start=(jt == 0), stop=(jt == JT - 1),
                                tile_position=(0, 32 * bh),
                            )
                            if final:
                                nc.tensor.matmul(
                                    ps_u[32 * bh:32 * bh + 32, 1, :],
                                    lhsT_hi, vl[:, bh, kt, :],
                                    start=(jt == 0), stop=False,
                                    tile_position=(0, 32 * bh),
                                )
                                nc.tensor.matmul(
                                    ps_u[32 * bh:32 * bh + 32, 1, :],
                                    attn_nT_lo[:, g, jt, 32 * bh:32 * bh + 32],
                                    vt[:, bh, kt, :],
                                    start=False, stop=(jt == JT - 1),
                                    tile_position=(0, 32 * bh),
                                )
                if not final:
                    coef = ALPHA / (1.0 - ALPHA) ** (t + 1)
                    rc = smallp.tile([128, 1], F32, tag="rc")
                    nc.vector.tensor_scalar_mul(rc[:], recip_sb[:, g:g + 1], coef)
                    nc.vector.scalar_tensor_tensor(
                        slots_ab[(t + 1) % 2][:, g, :], ps_u[:, 0, :], rc[:],
                        slots_ab[t % 2][:, g, :],
                        op0=ALU.mult, op1=ALU.add)
                else:
                    tu = chunkp.tile([128, 512], BF16, tag="tu")
                    nc.scalar.activation(tu[:], ps_u[:, 1, :], AF.Copy,
                                         scale=ISLO)
                    nc.vector.tensor_add(slots_ab[1][:, g, :], ps_u[:, 0, :],
                                         tu[:])
                    nc.vector.tensor_scalar(
                        slots_ab[1][:, g, :], slots_ab[1][:, g, :],
                        recip_sb[:, g:g + 1], None, op0=ALU.mult)

            def emit_iter(t):
                emit_q_phase(t)
                ac128 = emit_attn_pass1(t)
                for g in range(G):
                    for nt in range(NT):
                        emit_attn_chunk2(t, g, nt, ac128)
                nc.vector.tensor_reduce(rs_sb[:], rs_parts[:], axis=AX.X,
                                        op=ALU.add)
                nc.vector.tensor_scalar_add(recip_sb[:], rs_sb[:], EPS)
                nc.vector.reciprocal(recip_sb[:], recip_sb[:])
                for g in range(G):
                    emit_update_group(t, g)

            for rep in range(repeat):
                if rep > 0:
                    for g in range(G):
                        nc.scalar.dma_start(slots_ab[0][:, g, :], m128_h[:])
                for t in range(ITERS):
                    emit_iter(t)

            for g in range(G):
                nc.scalar.dma_start(out_h[g], slots_ab[1][:, g, :])

    nc.compile()
    _CACHE[key] = nc
    return nc


def _prep_inputs(x, meta_embed, Wq, g1, b1, g2, b2):
    bf16 = ml_dtypes.bfloat16
    f8 = ml_dtypes.float8_e4m3
    x3 = np.asarray(x, dtype=np.float32).reshape(B, C, HW)
    Wq = np.asarray(Wq, dtype=np.float32)
    meta = np.asarray(meta_embed, dtype=np.float32)
    wq_t = np.ascontiguousarray(
        Wq.T.reshape(DT, 128, C).transpose(1, 0, 2)).astype(bf16)
    mT_t = np.ascontiguousarray(
        meta.T.reshape(DT, 128, N).transpose(1, 0, 2)).astype(bf16)
    m128 = np.ascontiguousarray(np.tile(meta, (BH, 1))).astype(np.float32)
    g1b1 = np.stack([np.asarray(g1, np.float32).reshape(DT, 128).T,
                     np.asarray(b1, np.float32).reshape(DT, 128).T], axis=-1)
    g1b1 = np.ascontiguousarray(g1b1)
    g2b2 = np.stack([np.tile(np.asarray(g2, np.float32), BH),
                     np.tile(np.asarray(b2, np.float32), BH)], axis=-1)
    g2b2 = np.ascontiguousarray(g2b2)
    fold = np.ascontiguousarray(np.tile(np.eye(N, dtype=np.float32), (BH, BH)))

    in_maps = []
    for c in range(N_CORES):
        sl = x3[c * B_LOC:(c + 1) * B_LOC]          # [8, 512, 4096] f32
        hi = sl.astype(f8)
        hif = hi.astype(np.float32)
        lo = ((sl - hif) * SLO).astype(f8)
        # xnhi [NT, 128, B_LOC, DTP, 1024]:
        #   (nt,p,b,dtp,kt*512+w) = hi[b, (2dtp+kt)*128+p, nt*512+w]
        xnhi = np.ascontiguousarray(
            hi.reshape(B_LOC, DTP, 2, 128, NT, 512).transpose(4, 3, 0, 1, 2, 5)
        ).reshape(NT, 128, B_LOC, DTP, 1024)
        # xnlo [NT, DTP, G, 128, BH, 2, 512]:
        #   (nt,dtp,g,p,bh,kt,w) = lo[4g+bh, dtp*256+kt*128+p, nt*512+w]
        xnlo = np.ascontiguousarray(
            lo.reshape(G, BH, DTP, 2, 128, NT, 512).transpose(5, 2, 0, 4, 1, 3, 6))
        # xt [G, JTP, 128, BH, 2, 512]:
        #   (g,jtp,p,bh,kt,dd) = v[4g+bh, dd, jtp*256+kt*128+p]
        xthi = np.ascontiguousarray(
            hi.reshape(G, BH, 512, JTP, 2, 128).transpose(0, 3, 5, 1, 4, 2))
        xtlo = np.ascontiguousarray(
            lo.reshape(G, BH, 512, JTP, 2, 128).transpose(0, 3, 5, 1, 4, 2))
        in_maps.append({
            "xnhi": xnhi, "xnlo": xnlo, "xthi": xthi, "xtlo": xtlo,
            "wq": wq_t, "mT": mT_t, "m128": m128,
            "g1b1": g1b1, "g2b2": g2b2, "fold": fold,
        })
    return in_maps


def _unpack(results):
    out = np.empty((B, N, C), dtype=np.float32)
    for c in range(N_CORES):
        r = results[c]["out"]          # [G, 128, C]
        for g in range(G):
            blk = r[g].reshape(BH, N, C)
            for bh in range(BH):
                out[c * B_LOC + g * BH + bh] = blk[bh]
    return out


def run(trace=False, **inputs):
    nc = _build()
    in_maps = _prep_inputs(
        inputs["x"], inputs["meta_embed"], inputs["Wq"],
        inputs["g1"], inputs["b1"], inputs["g2"], inputs["b2"])
    res = run_bass_kernel_spmd(nc, in_maps, core_ids=list(range(N_CORES)),
                               trace=trace)
    return _unpack(res.results), res


def kernel(**inputs):
    out, _ = run(trace=False, **inputs)
    return out
